# revision 3
# baseline (speedup 1.0000x reference)
"""Trainium2 Bass kernel for a DecoderRNN (embedding -> 24-step LSTM -> vocab projection).

Shapes (hardcoded): B=128, T=24, H=E=1024, V=32000, 8 NeuronCores.

v5 design (v3 = staged baseline, v4 = remote-DMA experiment):
  Keeps v3's per-step 32KB AllGather for the h-exchange (measured ~13.5us
  per chained round on this stack -- remote SBUF->SBUF DMA was ~4x slower
  for data, its descriptors cost ~1us per 256B partition), but removes
  everything else from the recurrence-critical path:
  - The LSTM cell runs in TRANSPOSED layout [h_local=128, batch=128]:
    gate matmuls produce [gate_col, b] with W chunks stationary, so h_new
    is directly the publish slice. No PE transpose, no XBAR transpose
    reads -- the gather output is consumed with plain contiguous DMAs.
  - Gate matmuls accumulate onto the step's input-projection PSUM tile
    (xp precomputed 2 steps ahead, gate bias folded into its seed), so no
    identity re-seed and no xp round-trip through SBUF f16.
  - Gate matmuls iterate chunk-outer so they can start as soon as chunk 0
    of the gather lands (reads spread over 4 DMA rings).
  - The vocab bias is added on the DVE (broadcast add) instead of a
    500-cycle PE seed matmul per tile: saves 4k PE cycles/step.
  - The vocab projection (one step behind) and xp prefetch fill the PE
    during the collective window, as in v3.
"""

import numpy as np

import concourse.bass as bass
import concourse.tile as tile
import concourse.mybir as mybir
from concourse import bacc
from concourse.bass_utils import run_bass_kernel_spmd, axon_active

B, T = 128, 24
H, E, V = 1024, 1024, 32000
NCORES = 8
VSH = V // NCORES          # 4000 vocab columns per core
VT = 500                   # projection N-tile (8 per core)
KT = H // 128              # 8 contraction chunks

F32 = mybir.dt.float32
F16 = mybir.dt.float16

VOCAB_BIAS_ON_DVE = True
# ablation switches (timing experiments only -- breaks numerics when set)
ABL_NO_VOCAB = False
ABL_NO_EXCHANGE = False
ABL_NO_STORES = False

_CACHE = {}


def _lstm_body(nc, tc, tensors, n_iter):
    (xT_all, wih, whh, gb4, ind4, w_outT, b_out, c0, h0T,
     out_c, dram) = tensors
    NT = VSH // VT
    TT = T * n_iter
    # ring roles: sync+scalar carry the latency-critical bounce/gather
    # traffic; the bulky out_c stores ride the gpsimd ring (issued after
    # each step's collective in program order, so they never delay it)
    RINGS = [nc.sync, nc.scalar, nc.gpsimd]

    with tc.tile_pool(name="w", bufs=1) as w_p, \
         tc.tile_pool(name="xT", bufs=3) as xT_p, \
         tc.tile_pool(name="hT", bufs=2) as hT_p, \
         tc.tile_pool(name="tmp", bufs=4) as tmp_p, \
         tc.tile_pool(name="ob", bufs=3) as ob_p, \
         tc.tile_pool(name="g_ps", bufs=3, space="PSUM") as g_ps, \
         tc.tile_pool(name="c_ps", bufs=3, space="PSUM") as c_ps:
        # --- resident loads -------------------------------------------------
        wih_t = w_p.tile([128, KT, 512], F16)
        nc.sync.dma_start(wih_t[:], wih[:])
        whh_t = w_p.tile([128, KT, 512], F16)
        nc.sync.dma_start(whh_t[:], whh[:])
        # single-matmul bias seed operands: a start=True matmul zeroes the
        # ENTIRE PSUM bank (not just its output region), so the 4 gate
        # blocks' biases must be seeded by ONE matmul: out[m,(g,b)] =
        # sum_g' gb4[g',m] * ind4[g',(g,b)] = bias[g,m].
        gb4_t = w_p.tile([4, 128], F16)
        nc.sync.dma_start(gb4_t[:], gb4[:])
        ind4_t = w_p.tile([4, 512], F16)
        nc.sync.dma_start(ind4_t[:], ind4[:])
        c_st = w_p.tile([128, 128], F32)
        nc.sync.dma_start(c_st[:], c0[:])
        if VOCAB_BIAS_ON_DVE:
            bo = w_p.tile([128, VSH], F16)
            nc.sync.dma_start(bo[:], b_out[:])
        else:
            bo = w_p.tile([1, VSH], F16)
            nc.sync.dma_start(bo[:], b_out[0:1, :])
        h0T_t = w_p.tile([128, KT, 128], F16, tag="hT0", name="h0T_t")
        nc.scalar.dma_start(h0T_t[:], h0T[:])
        # W_out preload rides the store ring so steps 0-2's latency-critical
        # bounce/gather DMAs on sync/scalar are not queued behind 8MB
        wo = {}
        for n in range(NT):
            for k in range(KT):
                wt = w_p.tile([128, VT], F16, tag=f"wo{n}_{k}",
                              name=f"wo{n}_{k}")
                ring = nc.sync if (n * KT + k) % 2 == 0 else nc.scalar
                ring.dma_start(wt[:], w_outT[:, k, n * VT:(n + 1) * VT])
                wo[(n, k)] = wt

        xp = {}

        def xp_pre(t):
            """xp_ps[t] = bias + x_t W_ih^T in [gate_col, b] layout; the
            step-t gate matmuls accumulate h W_hh^T on top of this tile."""
            xT_t = xT_p.tile([128, KT, 128], F16, tag="xT", name="xT_t")
            nc.scalar.dma_start(xT_t[:], xT_all[t % T, :, :, :])
            ps = g_ps.tile([128, 4, 128], F32, tag="ps", name="xp_ps")
            nc.tensor.matmul(ps[:], gb4_t[:, :], ind4_t[:, :],
                             start=True, stop=False)
            for k in range(KT):
                for g in range(4):
                    gs = slice(g * 128, (g + 1) * 128)
                    nc.tensor.matmul(ps[:, g, :], wih_t[:, k, gs],
                                     xT_t[:, k, :], start=False, stop=False)
            xp[t] = ps

        # ~1MB of logit stores per step: a single HWDGE queue drains at only
        # ~31GB/s, so spread them over all three DMA-capable queues
        STORE_RINGS = [nc.gpsimd, nc.gpsimd, nc.scalar,
                       nc.scalar, nc.scalar, nc.sync, nc.sync, nc.gpsimd]

        def vocab_tiles(t, hT_src):
            if ABL_NO_VOCAB:
                return
            for n in range(NT):
                ns = slice(n * VT, (n + 1) * VT)
                ps = c_ps.tile([128, VT], F32, tag="ps", name="v_ps")
                for k in range(KT):
                    nc.tensor.matmul(ps[:], hT_src[:, k, :], wo[(n, k)][:, :],
                                     start=(VOCAB_BIAS_ON_DVE and k == 0),
                                     stop=(k == KT - 1))
                obn = ob_p.tile([128, VT], F16, tag="ob", name="ob")
                if VOCAB_BIAS_ON_DVE:
                    nc.vector.tensor_add(obn[:], ps[:], bo[:, ns])
                else:
                    nc.scalar.activation(obn[:], ps[:],
                                         mybir.ActivationFunctionType.Copy)
                if not ABL_NO_STORES or n == 0:
                    STORE_RINGS[n].dma_start(out_c[:, t % T, ns], obn[:])

        xp_pre(0)
        xp_pre(1)

        hT_prev = h0T_t
        for t in range(TT):
            # --- gate accumulation (chunk-outer: consume gather chunks as
            # they land) ----------------------------------------------------
            ps = xp.pop(t)
            for u in range(KT):
                for g in range(4):
                    gs = slice(g * 128, (g + 1) * 128)
                    nc.tensor.matmul(ps[:, g, :], whh_t[:, u, gs],
                                     hT_prev[:, u, :],
                                     start=False, stop=(u == KT - 1))
            # --- activations + cell (transposed [h_local, b]) ---------------
            sif = tmp_p.tile([128, 3, 128], F32, tag="sif", name="sif")
            nc.scalar.activation(sif[:], ps[:, 0:3, :],
                                 mybir.ActivationFunctionType.Sigmoid)
            tg = tmp_p.tile([128, 128], F32, tag="tg", name="tg")
            nc.scalar.activation(tg[:], ps[:, 3, :],
                                 mybir.ActivationFunctionType.Tanh)
            ig = tmp_p.tile([128, 128], F32, tag="ig", name="ig")
            nc.vector.tensor_mul(ig[:], sif[:, 0, :], tg[:])
            fc = tmp_p.tile([128, 128], F32, tag="fc", name="fc")
            nc.vector.tensor_mul(fc[:], sif[:, 1, :], c_st[:])
            nc.vector.tensor_add(c_st[:], ig[:], fc[:])
            tnh = tmp_p.tile([128, 128], F32, tag="tnh", name="tnh")
            nc.scalar.activation(tnh[:], c_st[:],
                                 mybir.ActivationFunctionType.Tanh)
            h_new = tmp_p.tile([128, 128], F16, tag="hn", name="h_new")
            nc.vector.tensor_mul(h_new[:], sif[:, 2, :], tnh[:])
            # --- publish + gather -------------------------------------------
            if ABL_NO_EXCHANGE:
                if t > 0:
                    vocab_tiles(t - 1, hT_prev)
                if t + 2 < TT:
                    xp_pre(t + 2)
                hcp = hT_p.tile([128, KT, 128], F16, tag="hT", name="hT_cur")
                for u in range(KT):
                    nc.vector.tensor_copy(hcp[:, u, :], h_new[:])
                hT_prev = hcp
                continue
            bounce = dram.tile([128, 128], F16, tag=f"hbd{t}")
            hg = dram.tile([NCORES, 128, 128], F16, tag=f"hg{t}",
                           addr_space="Shared", name=f"hg{t}")
            nc.sync.dma_start(bounce[:], h_new[:])
            cc = nc.gpsimd.collective_compute(
                "AllGather", mybir.AluOpType.bypass,
                ins=[bounce.opt()], outs=[hg.opt()],
                replica_groups=[list(range(NCORES))])
            hT_cur = hT_p.tile([128, KT, 128], F16, tag="hT", name="hT_cur")
            for u in range(KT):
                d = RINGS[u % 3].dma_start(hT_cur[:, u, :], hg[u, :, :])
                tile.add_dep_helper(d.ins, cc.ins, sync=True,
                                    reason="hT read after AllGather")
            # --- PE fillers -------------------------------------------------
            if t > 0:
                vocab_tiles(t - 1, hT_prev)
            if t + 2 < TT:
                xp_pre(t + 2)
            hT_prev = hT_cur
        vocab_tiles(TT - 1, hT_prev)


def _build(variant="full", n_iter=1):
    nc = bacc.Bacc("TRN2", target_bir_lowering=False, debug=False)

    xT_all = nc.dram_tensor("xT_all", [T, 128, KT, 128], F16, kind="ExternalInput")
    wih = nc.dram_tensor("wih", [128, KT, 512], F16, kind="ExternalInput")
    whh = nc.dram_tensor("whh", [128, KT, 512], F16, kind="ExternalInput")
    gb4 = nc.dram_tensor("gb4", [4, 128], F16, kind="ExternalInput")
    ind4 = nc.dram_tensor("ind4", [4, 512], F16, kind="ExternalInput")
    w_outT = nc.dram_tensor("w_outT", [128, KT, VSH], F16, kind="ExternalInput")
    b_out = nc.dram_tensor("b_out", [128, VSH], F16, kind="ExternalInput")
    c0 = nc.dram_tensor("c0", [128, 128], F32, kind="ExternalInput")
    h0T = nc.dram_tensor("h0T", [128, KT, 128], F16, kind="ExternalInput")
    out_c = nc.dram_tensor("out_c", [B, T, VSH], F16, kind="ExternalOutput")

    if variant == "null":
        with tile.TileContext(nc) as tc:
            with tc.tile_pool(name="p", bufs=2) as pool:
                t0 = pool.tile([128, VT], F16)
                nc.sync.dma_start(t0[:], w_outT[:, 0, 0:VT])
                for t in range(T):
                    nc.sync.dma_start(out_c[:, t, 0:VT], t0[:])
        nc.compile()
        return nc

    with tile.TileContext(nc) as tc:
        with tc.tile_pool(name="dram", bufs=1, space="DRAM") as dram:
            _lstm_body(nc, tc, (xT_all, wih, whh, gb4, ind4, w_outT,
                                b_out, c0, h0T, out_c, dram), n_iter)
    nc.compile()
    return nc


def _prep_inputs(features, captions, emb, W_ih, W_hh, b_ih, b_hh, W_out, b_out):
    features = np.asarray(features, np.float32)
    captions = np.asarray(captions)
    emb = np.asarray(emb, np.float32)
    W_ih = np.asarray(W_ih, np.float32)
    W_hh = np.asarray(W_hh, np.float32)
    b_ih = np.asarray(b_ih, np.float32)
    b_hh = np.asarray(b_hh, np.float32)
    W_out = np.asarray(W_out, np.float32)
    b_out = np.asarray(b_out, np.float32)

    x = emb[captions]                               # [B, T, E] host gather
    xT_all = (x.transpose(1, 2, 0)
                .reshape(T, KT, 128, B)
                .transpose(0, 2, 1, 3)).astype(np.float16)  # [T, e, k, b]
    gb = b_ih + b_hh
    fT = features.T                                 # [H, B]
    h0T = np.ascontiguousarray(
        fT.reshape(KT, 128, B).transpose(1, 0, 2)).astype(np.float16)

    common = {
        "xT_all": xT_all,
        "h0T": h0T,
        "ind4": np.kron(np.eye(4, dtype=np.float16),
                        np.ones((1, 128), np.float16)),
    }
    per_core = []
    for c in range(NCORES):
        vs = slice(c * VSH, (c + 1) * VSH)
        # gate blocks [i|f|o|g] so one sigmoid covers i,f,o contiguously
        rows = np.concatenate(
            [np.arange(g * H + c * 128, g * H + (c + 1) * 128)
             for g in (0, 1, 3, 2)])
        wih_c = np.ascontiguousarray(
            W_ih[rows].reshape(512, KT, 128).transpose(2, 1, 0)
        ).astype(np.float16)
        whh_c = np.ascontiguousarray(
            W_hh[rows].reshape(512, KT, 128).transpose(2, 1, 0)
        ).astype(np.float16)
        per_core.append({
            "wih": wih_c,
            "whh": whh_c,
            "gb4": gb[rows].astype(np.float16).reshape(4, 128),
            "w_outT": W_out[vs].reshape(VSH, KT, 128).transpose(2, 1, 0)
                           .astype(np.float16),
            "b_out": np.ascontiguousarray(
                np.broadcast_to(b_out[vs].astype(np.float16), (128, VSH))),
            "c0": np.ascontiguousarray(fT[c * 128:(c + 1) * 128, :]),
        })
    return common, per_core


# ---------------------------------------------------------------------------
# axon runner: cached jit + device-resident uploads (same as baseline v3)
# ---------------------------------------------------------------------------

class _AxonRunner:
    def __init__(self, nc):
        import jax
        from jax.sharding import Mesh, PartitionSpec, NamedSharding
        from jax.experimental.shard_map import shard_map
        from concourse import bass2jax

        bass2jax.install_neuronx_cc_hook()
        self.jax = jax
        self.nc = nc
        partition_name = (nc.partition_id_tensor.name
                          if nc.partition_id_tensor else None)
        in_names, out_names, out_avals = [], [], []
        for alloc in nc.m.functions[0].allocations:
            if not isinstance(alloc, mybir.MemoryLocationSet):
                continue
            name = alloc.memorylocations[0].name
            if alloc.kind == "ExternalInput":
                if name != partition_name:
                    in_names.append(name)
            elif alloc.kind == "ExternalOutput":
                out_names.append(name)
                out_avals.append(jax.core.ShapedArray(
                    tuple(alloc.tensor_shape), mybir.dt.np(alloc.dtype)))
        self.in_names = list(in_names)
        self.out_names = out_names
        n_params = len(in_names)
        cfg_in_names = in_names + out_names
        if partition_name is not None:
            cfg_in_names.append(partition_name)

        def _body(*args):
            operands = list(args)
            if partition_name is not None:
                operands.append(bass2jax.partition_id_tensor())
            outs = bass2jax._bass_exec_p.bind(
                *operands,
                out_avals=tuple(out_avals),
                in_names=tuple(cfg_in_names),
                out_names=tuple(out_names),
                lowering_input_output_aliases=(),
                sim_require_finite=True,
                sim_require_nnan=True,
                nc=nc,
            )
            return tuple(outs)

        devices = jax.devices()[:NCORES]
        self.mesh = Mesh(np.asarray(devices), ("core",))
        self.sharding = NamedSharding(self.mesh, PartitionSpec("core"))
        n_outs = len(out_names)
        self.fn = jax.jit(
            shard_map(_body, mesh=self.mesh,
                      in_specs=(PartitionSpec("core"),) * (n_params + n_outs),
                      out_specs=(PartitionSpec("core"),) * n_outs,
                      check_rep=False),
            keep_unused=True,
        )
        self.zeros = [
            jax.jit(lambda a=a: jax.numpy.zeros((NCORES * a.shape[0], *a.shape[1:]),
                                                a.dtype),
                    out_shardings=self.sharding)()
            for a in out_avals
        ]
        self.out_avals = out_avals
        self._upload_cache = {}

    def _resident(self, name, arrs):
        key = tuple(id(a) for a in arrs)
        sig = []
        for a in arrs:
            flat = a.reshape(-1)
            stride = max(1, flat.shape[0] // 997)
            sig.append(float(np.asarray(flat[::stride], np.float64).sum()))
        sig = tuple(sig)
        hit = self._upload_cache.get(name)
        if hit is not None and hit[0] == key and hit[1] == sig:
            return hit[3]
        cat = np.concatenate(arrs, axis=0)
        buf = self.jax.device_put(cat, self.sharding)
        self._upload_cache[name] = (key, sig, list(arrs), buf)
        return buf

    def run(self, common, per_core):
        dev_in = []
        for name in self.in_names:
            if name in common:
                arrs = [common[name]] * NCORES
            else:
                arrs = [pc[name] for pc in per_core]
            dev_in.append(self._resident(name, arrs))
        outs = self.fn(*dev_in, *self.zeros)
        self.jax.block_until_ready(outs)
        return [
            {name: np.asarray(outs[i]).reshape(NCORES, *self.out_avals[i].shape)[c]
             for i, name in enumerate(self.out_names)}
            for c in range(NCORES)
        ]


def _run(nc, common, per_core, key="runner"):
    if axon_active():
        if key not in _CACHE:
            _CACHE[key] = _AxonRunner(nc)
        return _CACHE[key].run(common, per_core)
    in_maps = [dict(common, **pc) for pc in per_core]
    res = run_bass_kernel_spmd(nc, in_maps, core_ids=list(range(NCORES)))
    return res.results


def kernel(**inputs) -> np.ndarray:
    common, per_core = _prep_inputs(**inputs)
    if "full" not in _CACHE:
        _CACHE["full"] = _build("full")
    nc = _CACHE["full"]

    results = _run(nc, common, per_core)

    out = np.empty((B, T + 1, V), np.float32)
    out[:, 0, :] = 0.0
    out[:, 0, 1] = 1.0
    for c in range(NCORES):
        out[:, 1:, c * VSH:(c + 1) * VSH] = results[c]["out_c"]
    return out
